# revision 37
# baseline (speedup 1.0000x reference)
"""Decoupled InfoNCE loss on 8 Trainium2 NeuronCores (Bass/Tile SPMD).

Math (reference):
    e = x / max(||x||, 1e-8);  sim = (e @ e.T) / 0.1
    pos = (t_i == t_j);  lse_neg = LSE_j(sim | not pos);  lse_pos = LSE_j(sim | pos & j != i)
    loss = sum_i (lse_neg_i - lse_pos_i)

Device strategy (per core c, anchors = rows [c*B, c*B+B)):
  * All logits sim/T lie in [-10, 10], so exp never overflows and the LSE
    max-subtraction can be dropped: lse = log(sum exp(sim/T)).
  * Inputs are row-rotated per core (np.roll) so each core's anchors are rows
    [0, B) of its own copy -> all 8 cores run one identical static program.
  * Normalization is split to keep it off the critical path: the j-side
    1/||x_j|| is applied as the per-partition `scale` AP of the Exp
    activation, so eT8 = transpose(RAW x) in fp8-e4m3 (two K-plane slabs
    [128, 2, n]) depends only on the input DMA. Only the 8 anchor tiles get
    pre-scaled (eA8 = 10 * inv_i * x_i). One DoubleRow matmul per (j-block,
    anchor-block) contracts K=256 at 0.5 PE cycles/row.
  * exp tiles are fp8-e5m2 (range to 57344 covers exp(10) unscaled) stored as
    jb-parity pairs [128, 2, 1024]; the rank-64 class-mask accumulation
    M[cls, i] = sum_j 1[t_j==cls] * exp(sim_ji) also runs as DoubleRow fp8
    matmuls over paired one-hot tags. Then
      S_pos_incl[i] = M[t_i, i],  S_neg[i] = sum_cls M[cls, i] - S_pos_incl
    by elementwise one-hot select + ones-matmul column sums (exact fp32).
  * The diagonal exp(sim_ii) is read back bit-exactly from the same fp8 exp
    tiles (first 8 j-blocks hold the diagonal after rotation), so subtracting
    it from S_pos_incl cancels exactly despite coarse e5m2 rounding of e^10.
  * Engine split: PE = bf16 transposes + DoubleRow fp8 sims/tags; Act = sqrt
    batched 8 tiles at a time, one [128, 1024] Exp per j-block, final ln;
    DVE = squares + row-sum reductions, PSUM->SBUF fp8 cast copies, phase-3
    selects; Pool = diagonal-extract multiplies. x arrives host-pre-tiled
    [128, n/128, d] bf16 so each 8-tile group is one contiguous-per-partition
    DMA; phase-1 groups, anchor build, and j-blocks are emitted zero-lag so
    the pipeline drains with a single trailing block.
  * Output per core: per-anchor loss rows [B]; host concatenates and sums.
"""

import sys

if "/opt/trn_rl_repo" not in sys.path:
    sys.path.insert(0, "/opt/trn_rl_repo")

import numpy as np

N = 8192          # total rows
D = 256           # embedding dim
C = 64            # num classes
NCORES = 8
B = N // NCORES   # anchors per core
SQT = float(np.sqrt(10.0))  # sqrt(1/temperature); applied to both operands
EPS = 1e-8

_NC_CACHE = {}


def _build_nc(n=N, d=D, ncls=C, ncores=NCORES, reps=1):
    import concourse.bass as bass
    import concourse.mybir as mybir
    from concourse import tile

    f32 = mybir.dt.float32
    bf16 = mybir.dt.bfloat16
    f8e4 = mybir.dt.float8e4
    f8e5 = mybir.dt.float8e5
    Act = mybir.ActivationFunctionType

    b = n // ncores       # anchors per core
    nt = n // 128         # j tiles
    hb = d // 128         # K planes
    nab = b // 512        # 512-wide anchor blocks
    ndj = b // 128        # j-blocks containing diagonal (first ndj blocks)
    npair = nt // 2       # DoubleRow jb pairs
    G = 8                 # row tiles per DMA/norm group

    nc = bass.Bass()
    # x pre-tiled on host to [128, n/128, d]: x_t[p, t, :] = x[t*128 + p, :],
    # so each partition's group slice is one contiguous DMA descriptor.
    # Shipped as bf16: the PE transposes consume it directly (fp8 transpose
    # needs stride-2 outputs, so the fp8 cast happens in the PSUM->SBUF copy)
    # and the row norms are computed from the same values.
    x_d = nc.dram_tensor("x", [128, n // 128, d], bf16, kind="ExternalInput")
    tagp_d = nc.dram_tensor("tagp", [128, npair, 2, ncls], f8e5, kind="ExternalInput")
    oha_d = nc.dram_tensor("oha", [ncls, b], f32, kind="ExternalInput")
    i128_d = nc.dram_tensor("i128", [128, 128], f32, kind="ExternalInput")
    i128b_d = nc.dram_tensor("i128b", [128, 128], bf16, kind="ExternalInput")
    i8e5_d = nc.dram_tensor("i8e5", [128, 128], f8e5, kind="ExternalInput")
    loss_d = nc.dram_tensor("loss", [1, b], f32, kind="ExternalOutput")

    with tile.TileContext(nc) as tc:
        with (
            tc.tile_pool(name="persist", bufs=1) as pp,
            tc.tile_pool(name="work", bufs=10) as wp,
            tc.tile_pool(name="expp", bufs=4) as ep,
            tc.tile_pool(name="c0p", bufs=2, space="PSUM") as c0p,
            tc.tile_pool(name="smp", bufs=2, space="PSUM") as smp,
            tc.tile_pool(name="mp", bufs=1, space="PSUM") as mp,
        ):
            # ---- persistent SBUF ----
            eT8 = pp.tile([128, hb, n], f8e4, tag="eT8")    # raw x, transposed
            eA8 = pp.tile([128, hb, b], f8e4, tag="eA8")    # 10*inv_i*x anchors
            tagP = pp.tile([128, npair, 2, ncls], f8e5, tag="tagP")
            ohaS = pp.tile([ncls, b], f32, tag="ohaS")
            i128 = pp.tile([128, 128], f32, tag="i128")
            i128b = pp.tile([128, 128], bf16, tag="i128b")
            i8e5 = pp.tile([128, 128], f8e5, tag="i8e5")
            ones = pp.tile([ncls, 1], f32, tag="ones")
            ssA = pp.tile([128, nt], f32, tag="ssA")      # sum(x^2) per row tile
            nrmA = pp.tile([128, nt], f32, tag="nrmA")
            invA = pp.tile([128, nt], f32, tag="invA")
            ddrow = pp.tile([1, b], f32, tag="ddrow")     # exp(sim_ii)
            sposr = pp.tile([1, b], f32, tag="sposr")     # S_pos incl diag
            snegr = pp.tile([1, b], f32, tag="snegr")     # S_neg
            lnegr = pp.tile([1, b], f32, tag="lnegr")
            lossr = pp.tile([1, b], f32, tag="lossr")

            nc.vector.memset(ones[:], 1.0)

            # macc psum accumulators live across the whole pair loop
            macc = [mp.tile([ncls, 512], f32, tag=f"m{ab}", name=f"macc{ab}")
                    for ab in range(nab)]

            def emit_consts():
                # emitted after the first x-group DMA so the critical first
                # row tiles aren't queued behind half a megabyte of tables
                nc.sync.dma_start(out=i128b[:], in_=i128b_d[:])
                nc.sync.dma_start(out=i8e5[:], in_=i8e5_d[:])
                nc.sync.dma_start(out=i128[:], in_=i128_d[:])
                nc.sync.dma_start(out=tagP[:], in_=tagp_d[:])
                nc.sync.dma_start(out=ohaS[:], in_=oha_d[:])

            # reps>1 repeats the whole computation in one NEFF; used only to
            # measure per-iteration HW time as a slope (dispatch overhead on
            # the axon path dwarfs a single run).
            for _rep in range(reps):
                _emit_body(nc, tile, mybir,
                           dict(locals(), emit_consts=emit_consts if _rep == 0
                                else None))

    _split_multi_waits(nc)
    nc.finalize()
    return nc


def _emit_body(nc, tile, mybir, env):
    f32 = mybir.dt.float32
    bf16 = mybir.dt.bfloat16
    f8e4 = mybir.dt.float8e4
    f8e5 = mybir.dt.float8e5
    Act = mybir.ActivationFunctionType
    AX = mybir.AxisListType
    DR = mybir.MatmulPerfMode.DoubleRow
    n, d, ncls, b = env["n"], env["d"], env["ncls"], env["b"]
    nt, hb, nab, ndj = env["nt"], env["hb"], env["nab"], env["ndj"]
    npair, G = env["npair"], env["G"]
    x_d, loss_d = env["x_d"], env["loss_d"]
    eT8, eA8 = env["eT8"], env["eA8"]
    tagP, ohaS, ones = env["tagP"], env["ohaS"], env["ones"]
    i128, i128b, i8e5 = env["i128"], env["i128b"], env["i8e5"]
    ssA, nrmA, invA = env["ssA"], env["nrmA"], env["invA"]
    ddrow, sposr, snegr, lnegr, lossr = (env["ddrow"], env["sposr"],
                                         env["snegr"], env["lnegr"],
                                         env["lossr"])
    wp, ep, c0p, smp = env["wp"], env["ep"], env["c0p"], env["smp"]
    macc = env["macc"]

    # ---- phase 1 (per G-tile group) ----
    # eT8 gets RAW fp8 x transposed (no norm dependency: the j-side 1/||x_j||
    # is applied later as the per-partition scale of the Exp activation).
    # Norms are computed alongside: ss -> sqrt -> max -> 1/x into invA.
    def phase1a(g):
        xg = wp.tile([128, G, d], bf16, tag="xg", bufs=2, name="xg")
        nc.sync.dma_start(out=xg[:], in_=x_d[:, g * G:(g + 1) * G, :])
        for k in range(G):
            t = g * G + k
            sq = wp.tile([128, d], bf16, tag="sq", name="sq")
            nc.vector.tensor_mul(sq[:], xg[:, k, :], xg[:, k, :])
            nc.vector.reduce_sum(ssA[:, t:t + 1], sq[:], axis=AX.X)
        lo, hi = g * G, (g + 1) * G
        nc.scalar.activation(nrmA[:, lo:hi], ssA[:, lo:hi], Act.Sqrt)
        nc.vector.tensor_scalar_max(nrmA[:, lo:hi], nrmA[:, lo:hi], EPS)
        nc.vector.reciprocal(invA[:, lo:hi], nrmA[:, lo:hi])
        return xg

    def phase1b(g, xg):
        for a in range(G // 4):  # halves of 4 tiles
            pts = [smp.tile([128, 512], bf16, tag="sm", name=f"pt{h}")
                   for h in range(hb)]
            for k4 in range(4):
                k = a * 4 + k4
                for h in range(hb):
                    nc.tensor.matmul(pts[h][:, k4 * 128:(k4 + 1) * 128],
                                     xg[:, k, h * 128:(h + 1) * 128], i128b[:],
                                     is_transpose=True, start=True, stop=True)
            base = (g * G + a * 4) * 128
            for h in range(hb):
                nc.vector.tensor_copy(eT8[:, h, base:base + 512], pts[h][:])

    def build_anchors(xg0):
        # anchor operand = 10 * inv_i * x_i in fp8, transposed into eA8
        for a in range(2):
            pts = [smp.tile([128, 512], bf16, tag="sm", name=f"ptA{h}")
                   for h in range(hb)]
            for k4 in range(4):
                k = a * 4 + k4
                xbA = wp.tile([128, d], bf16, tag="xbA", name="xbA")
                nc.vector.tensor_scalar(xbA[:], xg0[:, k, :], invA[:, k:k + 1],
                                        10.0,
                                        op0=mybir.AluOpType.mult,
                                        op1=mybir.AluOpType.mult)
                for h in range(hb):
                    nc.tensor.matmul(pts[h][:, k4 * 128:(k4 + 1) * 128],
                                     xbA[:, h * 128:(h + 1) * 128], i128b[:],
                                     is_transpose=True, start=True, stop=True)
            for h in range(hb):
                nc.vector.tensor_copy(eA8[:, h, a * 512:(a + 1) * 512],
                                      pts[h][:])

    # ---- phase 2: DoubleRow sim chunks -> exp -> DoubleRow class sums ----
    live = {}  # pair -> exp tile [128, 2, 1024] (parity plane per jb)
    exd = {}   # consumed pairs kept addressable for deferred diag extraction

    def consume(p):
        ex = live.pop(p)
        exd[p] = ex
        for ab in range(nab):
            nc.tensor.matmul(macc[ab][:], tagP[:, p, :, :],
                             ex[:, :, ab * 512:(ab + 1) * 512],
                             start=(p == 0), stop=(p == npair - 1),
                             perf_mode=mybir.MatmulPerfMode.DoubleRow,
                             skip_group_check=True)

    def extract_diag(jb):
        # diagonal of sim lives in columns [jb*128, (jb+1)*128) of ex pair
        # jb//2 plane jb%2; deferred a few blocks so it stays off the
        # group-boundary critical path (the ex tile lives until consume)
        p, q = divmod(jb, 2)
        ex = live.get(p) or exd[p]
        dtmp = wp.tile([128, 128], f32, tag="dtmp")
        nc.gpsimd.tensor_mul(dtmp[:], ex[:, q, jb * 128:(jb + 1) * 128],
                             i8e5[:])
        dcol = wp.tile([128, 1], f32, tag="dcol")
        nc.vector.reduce_sum(dcol[:], dtmp[:], axis=AX.X)
        drow = smp.tile([1, 512], f32, tag="sm", name="drow")
        nc.tensor.matmul(drow[:, :128], dcol[:], i128[:],
                         start=True, stop=True)
        nc.vector.tensor_copy(ddrow[:, jb * 128:(jb + 1) * 128],
                              drow[:, :128])

    def emit_jb(jb):
        p, q = divmod(jb, 2)
        if q == 0:
            live[p] = ep.tile([128, 2, b], f8e5, tag="exp", name=f"ex_{p}")
        ex = live[p]
        c0b = c0p.tile([128, b], f32, tag="c0b", name=f"c0b_{jb}")
        for ab in range(nab):
            nc.tensor.matmul(c0b[:, ab * 512:(ab + 1) * 512],
                             eT8[:, :, jb * 128:(jb + 1) * 128],
                             eA8[:, :, ab * 512:(ab + 1) * 512],
                             start=True, stop=True,
                             perf_mode=mybir.MatmulPerfMode.DoubleRow,
                             skip_group_check=True)
        # per-partition scale = 1/||x_j|| folds the j-side normalization
        # into the exp: exp((x_j . 10*inv_i*x_i) * inv_j)
        nc.scalar.activation(ex[:, q, :], c0b[:], Act.Exp,
                             scale=invA[:, jb:jb + 1])
        if jb < ndj:
            extract_diag(jb)
        if q == 1 and p >= 1:
            consume(p - 1)

    # Zero-lag interleave: jb needs eT8 tiles 0..7 (anchor columns, = group 0)
    # plus tile jb, so jb group g is emitted right after phase-1 group g.
    # The pipeline drains with a single trailing jb instead of a whole phase.
    ngrp = nt // G
    for g in range(ngrp):
        xg = phase1a(g)
        if g == 0 and env.get("emit_consts"):
            env["emit_consts"]()
        phase1b(g, xg)
        if g == 0:
            build_anchors(xg)
        for jb in range(g * G, (g + 1) * G):
            emit_jb(jb)
    consume(npair - 1)

    # ---- phase 3+4, per anchor block so the two chains overlap ----
    # S_pos path: x1 = M*onehot (PSUM read) -> ones-matmul -> - diag -> ln
    # S_all path: copy -> ones-matmul; S_neg = S_all - S_pos_incl -> ln
    for ab in range(nab):
        sl = slice(ab * 512, (ab + 1) * 512)
        x1 = wp.tile([ncls, 512], f32, tag="x1")
        nc.vector.tensor_mul(x1[:], macc[ab][:], ohaS[:, sl])
        msb = wp.tile([ncls, 512], f32, tag="msb")
        nc.vector.tensor_copy(msb[:], macc[ab][:])
        s1 = smp.tile([1, 512], f32, tag="sm", name="s1")
        nc.tensor.matmul(s1[:], ones[:], x1[:], start=True, stop=True)
        nc.vector.tensor_copy(sposr[:, sl], s1[:])
        s2 = smp.tile([1, 512], f32, tag="sm", name="s2")
        nc.tensor.matmul(s2[:], ones[:], msb[:], start=True, stop=True)
        nc.vector.tensor_copy(snegr[:, sl], s2[:])
        nc.vector.tensor_sub(snegr[:, sl], snegr[:, sl], sposr[:, sl])
        nc.vector.tensor_sub(sposr[:, sl], sposr[:, sl], ddrow[:, sl])
        nc.scalar.activation(lnegr[:, sl], snegr[:, sl], Act.Ln)
        nc.scalar.activation(lossr[:, sl], sposr[:, sl], Act.Ln)
        nc.vector.tensor_sub(lossr[:, sl], lnegr[:, sl], lossr[:, sl])
    nc.sync.dma_start(out=loss_d[:], in_=lossr[:])


def _split_multi_waits(nc):
    """Move extra semaphore waits onto standalone EventSemaphore carriers.

    The pinned walrus build only has one sync-wait slot per engine
    instruction ("Too many sync wait commands"), while the Tile scheduler
    happily attaches several. All waits here are monotonic sem-ge-imm, so
    waiting sequentially on the same engine is equivalent to waiting on the
    conjunction.
    """
    import concourse.mybir as mybir

    for fn in nc.m.functions:
        for blk in fn.blocks:
            out = []
            for inst in blk.instructions:
                si = inst.sync_info
                if si is not None and si.on_wait and len(si.on_wait) > 1 and all(
                    w.wait_mode == "sem-ge-imm" for w in si.on_wait
                ):
                    for w in si.on_wait[:-1]:
                        carrier = mybir.InstEventSemaphore(
                            name=f"I-{nc.next_id()}-waitsplit",
                            engine=inst.engine,
                            sync_info=mybir.SyncInfo(on_wait=[w], on_update=[]),
                        )
                        nc.inst_map[carrier.name] = carrier
                        out.append(carrier)
                    inst.sync_info = mybir.SyncInfo(
                        on_wait=[si.on_wait[-1]], on_update=si.on_update
                    )
                out.append(inst)
            blk.instructions[:] = out


def _get_nc():
    key = (N, D, C, NCORES)
    if key not in _NC_CACHE:
        _NC_CACHE[key] = _build_nc(*key)
    return _NC_CACHE[key]


def make_in_maps(embeddings, target, n=N, ncls=C, ncores=NCORES):
    import ml_dtypes

    b = n // ncores
    nt = n // 128
    npair = nt // 2
    emb = np.asarray(embeddings, dtype=np.float32).astype(ml_dtypes.bfloat16)
    tgt = np.asarray(target).astype(np.int64) % ncls
    onehot = np.eye(ncls, dtype=np.float32)[tgt]  # [n, ncls]
    i128 = np.eye(128, dtype=np.float32)
    i128b = i128.astype(ml_dtypes.bfloat16)
    i8e5 = i128.astype(ml_dtypes.float8_e5m2)
    in_maps = []
    for c in range(ncores):
        xs = np.ascontiguousarray(
            np.roll(emb, -c * b, axis=0).reshape(nt, 128, -1).transpose(1, 0, 2)
        )
        oh = np.roll(onehot, -c * b, axis=0)
        # paired tag layout: tagp[j, p, q, cls] = onehot[(2p+q)*128 + j, cls]
        tagp = np.ascontiguousarray(
            oh.reshape(npair, 2, 128, ncls).transpose(2, 0, 1, 3)
        ).astype(ml_dtypes.float8_e5m2)
        oha = np.ascontiguousarray(oh[:b].T)  # [ncls, b]
        in_maps.append({"x": xs, "tagp": tagp, "oha": oha,
                        "i128": i128, "i128b": i128b, "i8e5": i8e5})
    return in_maps


def kernel(embeddings, target):
    from concourse.bass_utils import run_bass_kernel_spmd

    nc = _get_nc()
    in_maps = make_in_maps(embeddings, target)
    res = run_bass_kernel_spmd(nc, in_maps, list(range(NCORES))).results
    loss = np.concatenate([np.asarray(res[c]["loss"]).ravel() for c in range(NCORES)])
    return np.float32(loss.sum())


# revision 40
# speedup vs baseline: 2.1095x; 2.1095x over previous
"""Decoupled InfoNCE loss on 8 Trainium2 NeuronCores (Bass/Tile SPMD).

Math (reference):
    e = x / max(||x||, 1e-8);  sim = (e @ e.T) / 0.1
    pos = (t_i == t_j);  lse_neg = LSE_j(sim | not pos);  lse_pos = LSE_j(sim | pos & j != i)
    loss = sum_i (lse_neg_i - lse_pos_i)

Device strategy (per core c, anchors = rows [c*B, c*B+B)):
  * All logits sim/T lie in [-10, 10], so exp never overflows and the LSE
    max-subtraction can be dropped: lse = log(sum exp(sim/T)).
  * Inputs are row-rotated per core (np.roll) so each core's anchors are rows
    [0, B) of its own copy -> all 8 cores run one identical static program.
  * Normalization is split to keep it off the critical path: the j-side
    1/||x_j|| is applied as the per-partition `scale` AP of the Exp
    activation, so eT8 = transpose(RAW x) in fp8-e4m3 (two K-plane slabs
    [128, 2, n]) depends only on the input DMA. Only the 8 anchor tiles get
    pre-scaled (eA8 = 10 * inv_i * x_i). One DoubleRow matmul per (j-block,
    anchor-block) contracts K=256 at 0.5 PE cycles/row.
  * exp tiles are fp8-e5m2 (range to 57344 covers exp(10) unscaled) stored as
    jb-parity pairs [128, 2, 1024]; the rank-64 class-mask accumulation
    M[cls, i] = sum_j 1[t_j==cls] * exp(sim_ji) also runs as DoubleRow fp8
    matmuls over paired one-hot tags. Then
      S_pos_incl[i] = M[t_i, i],  S_neg[i] = sum_cls M[cls, i] - S_pos_incl
    by elementwise one-hot select + ones-matmul column sums (exact fp32).
  * The diagonal exp(sim_ii) is read back bit-exactly from the same fp8 exp
    tiles (first 8 j-blocks hold the diagonal after rotation), so subtracting
    it from S_pos_incl cancels exactly despite coarse e5m2 rounding of e^10.
  * Engine split: PE = bf16 transposes + DoubleRow fp8 sims/tags; Act = sqrt
    batched 8 tiles at a time, one [128, 1024] Exp per j-block, final ln;
    DVE = squares + row-sum reductions, PSUM->SBUF fp8 cast copies, phase-3
    selects; Pool = diagonal-extract multiplies. x arrives host-pre-tiled
    [128, n/128, d] bf16 so each 8-tile group is one contiguous-per-partition
    DMA; phase-1 groups, anchor build, and j-blocks are emitted zero-lag so
    the pipeline drains with a single trailing block.
  * Output per core: per-anchor loss rows [B]; host concatenates and sums.
"""

import sys

if "/opt/trn_rl_repo" not in sys.path:
    sys.path.insert(0, "/opt/trn_rl_repo")

import numpy as np

N = 8192          # total rows
D = 256           # embedding dim
C = 64            # num classes
NCORES = 8
B = N // NCORES   # anchors per core
SQT = float(np.sqrt(10.0))  # sqrt(1/temperature); applied to both operands
EPS = 1e-8

_NC_CACHE = {}


def _build_nc(n=N, d=D, ncls=C, ncores=NCORES, reps=1):
    import concourse.bass as bass
    import concourse.mybir as mybir
    from concourse import tile

    f32 = mybir.dt.float32
    bf16 = mybir.dt.bfloat16
    f8e4 = mybir.dt.float8e4
    f8e5 = mybir.dt.float8e5
    Act = mybir.ActivationFunctionType

    b = n // ncores       # anchors per core
    nt = n // 128         # j tiles
    hb = d // 128         # K planes
    nab = b // 512        # 512-wide anchor blocks
    ndj = b // 128        # j-blocks containing diagonal (first ndj blocks)
    npair = nt // 2       # DoubleRow jb pairs
    G = 8                 # row tiles per DMA/norm group

    nc = bass.Bass()
    # x pre-tiled on host to [128, n/128, d]: x_t[p, t, :] = x[t*128 + p, :],
    # so each partition's group slice is one contiguous DMA descriptor.
    # Shipped as bf16: the PE transposes consume it directly (fp8 transpose
    # needs stride-2 outputs, so the fp8 cast happens in the PSUM->SBUF copy)
    # and the row norms are computed from the same values.
    x_d = nc.dram_tensor("x", [128, n // 128, d], bf16, kind="ExternalInput")
    tagp_d = nc.dram_tensor("tagp", [128, npair, 2, ncls], f8e5, kind="ExternalInput")
    oha_d = nc.dram_tensor("oha", [ncls, b], f32, kind="ExternalInput")
    i128_d = nc.dram_tensor("i128", [128, 128], f32, kind="ExternalInput")
    i128b_d = nc.dram_tensor("i128b", [128, 128], bf16, kind="ExternalInput")
    i8e5_d = nc.dram_tensor("i8e5", [128, 128], f8e5, kind="ExternalInput")
    loss_d = nc.dram_tensor("loss", [1, b], f32, kind="ExternalOutput")

    with tile.TileContext(nc) as tc:
        with (
            tc.tile_pool(name="persist", bufs=1) as pp,
            tc.tile_pool(name="work", bufs=10) as wp,
            tc.tile_pool(name="expp", bufs=4) as ep,
            tc.tile_pool(name="c0p", bufs=2, space="PSUM") as c0p,
            tc.tile_pool(name="smp", bufs=2, space="PSUM") as smp,
            tc.tile_pool(name="mp", bufs=1, space="PSUM") as mp,
        ):
            # ---- persistent SBUF ----
            eT8 = pp.tile([128, hb, n], f8e4, tag="eT8")    # raw x, transposed
            eA8 = pp.tile([128, hb, b], f8e4, tag="eA8")    # 10*inv_i*x anchors
            tagP = pp.tile([128, npair, 2, ncls], f8e5, tag="tagP")
            ohaS = pp.tile([ncls, b], f32, tag="ohaS")
            i128 = pp.tile([128, 128], f32, tag="i128")
            i128b = pp.tile([128, 128], bf16, tag="i128b")
            i8e5 = pp.tile([128, 128], f8e5, tag="i8e5")
            ones = pp.tile([ncls, 1], f32, tag="ones")
            ssA = pp.tile([128, nt], f32, tag="ssA")      # sum(x^2) per row tile
            nrmA = pp.tile([128, nt], f32, tag="nrmA")
            invA = pp.tile([128, nt], f32, tag="invA")
            ddrow = pp.tile([1, b], f32, tag="ddrow")     # exp(sim_ii)
            sposr = pp.tile([1, b], f32, tag="sposr")     # S_pos incl diag
            snegr = pp.tile([1, b], f32, tag="snegr")     # S_neg
            lnegr = pp.tile([1, b], f32, tag="lnegr")
            lossr = pp.tile([1, b], f32, tag="lossr")

            nc.vector.memset(ones[:], 1.0)

            # macc psum accumulators live across the whole pair loop
            macc = [mp.tile([ncls, 512], f32, tag=f"m{ab}", name=f"macc{ab}")
                    for ab in range(nab)]

            def emit_consts():
                # emitted after the first x-group DMA so the critical first
                # row tiles aren't queued behind half a megabyte of tables
                nc.sync.dma_start(out=i128b[:], in_=i128b_d[:])
                nc.sync.dma_start(out=i8e5[:], in_=i8e5_d[:])
                nc.sync.dma_start(out=i128[:], in_=i128_d[:])
                nc.sync.dma_start(out=tagP[:], in_=tagp_d[:])
                nc.sync.dma_start(out=ohaS[:], in_=oha_d[:])

            # reps>1 repeats the whole computation in one NEFF; used only to
            # measure per-iteration HW time as a slope (dispatch overhead on
            # the axon path dwarfs a single run).
            for _rep in range(reps):
                _emit_body(nc, tile, mybir,
                           dict(locals(), emit_consts=emit_consts if _rep == 0
                                else None))

    _split_multi_waits(nc)
    nc.finalize()
    return nc


def _emit_body(nc, tile, mybir, env):
    f32 = mybir.dt.float32
    bf16 = mybir.dt.bfloat16
    f8e4 = mybir.dt.float8e4
    f8e5 = mybir.dt.float8e5
    Act = mybir.ActivationFunctionType
    AX = mybir.AxisListType
    DR = mybir.MatmulPerfMode.DoubleRow
    n, d, ncls, b = env["n"], env["d"], env["ncls"], env["b"]
    nt, hb, nab, ndj = env["nt"], env["hb"], env["nab"], env["ndj"]
    npair, G = env["npair"], env["G"]
    x_d, loss_d = env["x_d"], env["loss_d"]
    eT8, eA8 = env["eT8"], env["eA8"]
    tagP, ohaS, ones = env["tagP"], env["ohaS"], env["ones"]
    i128, i128b, i8e5 = env["i128"], env["i128b"], env["i8e5"]
    ssA, nrmA, invA = env["ssA"], env["nrmA"], env["invA"]
    ddrow, sposr, snegr, lnegr, lossr = (env["ddrow"], env["sposr"],
                                         env["snegr"], env["lnegr"],
                                         env["lossr"])
    wp, ep, c0p, smp = env["wp"], env["ep"], env["c0p"], env["smp"]
    macc = env["macc"]

    # ---- phase 1 (per G-tile group) ----
    # eT8 gets RAW fp8 x transposed (no norm dependency: the j-side 1/||x_j||
    # is applied later as the per-partition scale of the Exp activation).
    # Norms (ss -> sqrt -> max -> 1/x into invA) are emitted one group AHEAD
    # of their use, with the squares on the otherwise-idle Pool engine, so
    # the whole chain hides behind the previous group's exp work.
    def dma_group(g):
        xg = wp.tile([128, G, d], bf16, tag="xg", bufs=2, name="xg")
        nc.sync.dma_start(out=xg[:], in_=x_d[:, g * G:(g + 1) * G, :])
        return xg

    def norms(g, xg):
        eng = nc.vector if g == 0 else nc.gpsimd
        for k in range(G):
            t = g * G + k
            sq = wp.tile([128, d], bf16, tag="sq", name="sq")
            eng.tensor_mul(sq[:], xg[:, k, :], xg[:, k, :])
            nc.vector.reduce_sum(ssA[:, t:t + 1], sq[:], axis=AX.X)
        lo, hi = g * G, (g + 1) * G
        nc.scalar.activation(nrmA[:, lo:hi], ssA[:, lo:hi], Act.Sqrt)
        nc.vector.tensor_scalar_max(nrmA[:, lo:hi], nrmA[:, lo:hi], EPS)
        nc.vector.reciprocal(invA[:, lo:hi], nrmA[:, lo:hi])

    def phase1b(g, xg):
        for a in range(G // 4):  # halves of 4 tiles
            pt = smp.tile([128, hb, 512], bf16, tag="sm", name="pt")
            for k4 in range(4):
                k = a * 4 + k4
                for h in range(hb):
                    nc.tensor.matmul(pt[:, h, k4 * 128:(k4 + 1) * 128],
                                     xg[:, k, h * 128:(h + 1) * 128], i128b[:],
                                     is_transpose=True, start=True, stop=True)
            base = (g * G + a * 4) * 128
            nc.vector.tensor_copy(eT8[:, :, base:base + 512], pt[:])

    def build_anchors(xg0):
        # anchor operand = 10 * inv_i * x_i in fp8, transposed into eA8
        for a in range(2):
            pt = smp.tile([128, hb, 512], bf16, tag="sm", name="ptA")
            for k4 in range(4):
                k = a * 4 + k4
                xbA = wp.tile([128, d], bf16, tag="xbA", name="xbA")
                nc.vector.tensor_scalar(xbA[:], xg0[:, k, :], invA[:, k:k + 1],
                                        10.0,
                                        op0=mybir.AluOpType.mult,
                                        op1=mybir.AluOpType.mult)
                for h in range(hb):
                    nc.tensor.matmul(pt[:, h, k4 * 128:(k4 + 1) * 128],
                                     xbA[:, h * 128:(h + 1) * 128], i128b[:],
                                     is_transpose=True, start=True, stop=True)
            nc.vector.tensor_copy(eA8[:, :, a * 512:(a + 1) * 512], pt[:])

    # ---- phase 2: DoubleRow sim chunks -> exp -> DoubleRow class sums ----
    live = {}  # pair -> exp tile [128, 2, 1024] (parity plane per jb)
    exd = {}   # consumed pairs kept addressable for deferred diag extraction

    def consume(p):
        ex = live.pop(p)
        exd[p] = ex
        for ab in range(nab):
            nc.tensor.matmul(macc[ab][:], tagP[:, p, :, :],
                             ex[:, :, ab * 512:(ab + 1) * 512],
                             start=(p == 0), stop=(p == npair - 1),
                             perf_mode=mybir.MatmulPerfMode.DoubleRow,
                             skip_group_check=True)

    def extract_diag(jb):
        # diagonal of sim lives in columns [jb*128, (jb+1)*128) of ex pair
        # jb//2 plane jb%2; deferred a few blocks so it stays off the
        # group-boundary critical path (the ex tile lives until consume)
        p, q = divmod(jb, 2)
        ex = live.get(p) or exd[p]
        dtmp = wp.tile([128, 128], f32, tag="dtmp")
        nc.gpsimd.tensor_mul(dtmp[:], ex[:, q, jb * 128:(jb + 1) * 128],
                             i8e5[:])
        dcol = wp.tile([128, 1], f32, tag="dcol")
        nc.vector.reduce_sum(dcol[:], dtmp[:], axis=AX.X)
        drow = smp.tile([1, 512], f32, tag="sm", name="drow")
        nc.tensor.matmul(drow[:, :128], dcol[:], i128[:],
                         start=True, stop=True)
        nc.vector.tensor_copy(ddrow[:, jb * 128:(jb + 1) * 128],
                              drow[:, :128])

    def emit_jb(jb):
        p, q = divmod(jb, 2)
        if q == 0:
            live[p] = ep.tile([128, 2, b], f8e5, tag="exp", name=f"ex_{p}")
        ex = live[p]
        c0b = c0p.tile([128, b], f32, tag="c0b", name=f"c0b_{jb}")
        for ab in range(nab):
            nc.tensor.matmul(c0b[:, ab * 512:(ab + 1) * 512],
                             eT8[:, :, jb * 128:(jb + 1) * 128],
                             eA8[:, :, ab * 512:(ab + 1) * 512],
                             start=True, stop=True,
                             perf_mode=mybir.MatmulPerfMode.DoubleRow,
                             skip_group_check=True)
        # per-partition scale = 1/||x_j|| folds the j-side normalization
        # into the exp: exp((x_j . 10*inv_i*x_i) * inv_j)
        nc.scalar.activation(ex[:, q, :], c0b[:], Act.Exp,
                             scale=invA[:, jb:jb + 1])
        if jb < ndj:
            extract_diag(jb)
        if q == 1 and p >= 1:
            consume(p - 1)

    # Zero-lag interleave: jb needs eT8 tiles 0..7 (anchor columns, = group 0)
    # plus tile jb, so jb group g is emitted right after phase-1 group g.
    # The pipeline drains with a single trailing jb instead of a whole phase.
    ngrp = nt // G
    xgs = {0: dma_group(0)}
    if env.get("emit_consts"):
        env["emit_consts"]()
    norms(0, xgs[0])
    for g in range(ngrp):
        if g + 1 < ngrp:
            xgs[g + 1] = dma_group(g + 1)
        phase1b(g, xgs[g])
        if g == 0:
            build_anchors(xgs[0])
        for jb in range(g * G, (g + 1) * G):
            emit_jb(jb)
        if g + 1 < ngrp:
            norms(g + 1, xgs[g + 1])
        del xgs[g]
    consume(npair - 1)

    # ---- phase 3+4, per anchor block so the two chains overlap ----
    # S_pos path: x1 = M*onehot (PSUM read) -> ones-matmul -> - diag -> ln
    # S_all path: copy -> ones-matmul; S_neg = S_all - S_pos_incl -> ln
    for ab in range(nab):
        sl = slice(ab * 512, (ab + 1) * 512)
        x1 = wp.tile([ncls, 512], f32, tag="x1")
        nc.vector.tensor_mul(x1[:], macc[ab][:], ohaS[:, sl])
        msb = wp.tile([ncls, 512], f32, tag="msb")
        nc.vector.tensor_copy(msb[:], macc[ab][:])
        s1 = smp.tile([1, 512], f32, tag="sm", name="s1")
        nc.tensor.matmul(s1[:], ones[:], x1[:], start=True, stop=True)
        nc.vector.tensor_copy(sposr[:, sl], s1[:])
        s2 = smp.tile([1, 512], f32, tag="sm", name="s2")
        nc.tensor.matmul(s2[:], ones[:], msb[:], start=True, stop=True)
        nc.vector.tensor_copy(snegr[:, sl], s2[:])
        nc.vector.tensor_sub(snegr[:, sl], snegr[:, sl], sposr[:, sl])
        nc.vector.tensor_sub(sposr[:, sl], sposr[:, sl], ddrow[:, sl])
        nc.scalar.activation(lnegr[:, sl], snegr[:, sl], Act.Ln)
        nc.scalar.activation(lossr[:, sl], sposr[:, sl], Act.Ln)
        nc.vector.tensor_sub(lossr[:, sl], lnegr[:, sl], lossr[:, sl])
    nc.sync.dma_start(out=loss_d[:], in_=lossr[:])


def _split_multi_waits(nc):
    """Move extra semaphore waits onto standalone EventSemaphore carriers.

    The pinned walrus build only has one sync-wait slot per engine
    instruction ("Too many sync wait commands"), while the Tile scheduler
    happily attaches several. All waits here are monotonic sem-ge-imm, so
    waiting sequentially on the same engine is equivalent to waiting on the
    conjunction.
    """
    import concourse.mybir as mybir

    for fn in nc.m.functions:
        for blk in fn.blocks:
            out = []
            for inst in blk.instructions:
                si = inst.sync_info
                if si is not None and si.on_wait and len(si.on_wait) > 1 and all(
                    w.wait_mode == "sem-ge-imm" for w in si.on_wait
                ):
                    for w in si.on_wait[:-1]:
                        carrier = mybir.InstEventSemaphore(
                            name=f"I-{nc.next_id()}-waitsplit",
                            engine=inst.engine,
                            sync_info=mybir.SyncInfo(on_wait=[w], on_update=[]),
                        )
                        nc.inst_map[carrier.name] = carrier
                        out.append(carrier)
                    inst.sync_info = mybir.SyncInfo(
                        on_wait=[si.on_wait[-1]], on_update=si.on_update
                    )
                out.append(inst)
            blk.instructions[:] = out


def _get_nc():
    key = (N, D, C, NCORES)
    if key not in _NC_CACHE:
        _NC_CACHE[key] = _build_nc(*key)
    return _NC_CACHE[key]


def make_in_maps(embeddings, target, n=N, ncls=C, ncores=NCORES):
    import ml_dtypes

    b = n // ncores
    nt = n // 128
    npair = nt // 2
    emb = np.asarray(embeddings, dtype=np.float32).astype(ml_dtypes.bfloat16)
    tgt = np.asarray(target).astype(np.int64) % ncls
    onehot = np.eye(ncls, dtype=np.float32)[tgt]  # [n, ncls]
    i128 = np.eye(128, dtype=np.float32)
    i128b = i128.astype(ml_dtypes.bfloat16)
    i8e5 = i128.astype(ml_dtypes.float8_e5m2)
    in_maps = []
    for c in range(ncores):
        xs = np.ascontiguousarray(
            np.roll(emb, -c * b, axis=0).reshape(nt, 128, -1).transpose(1, 0, 2)
        )
        oh = np.roll(onehot, -c * b, axis=0)
        # paired tag layout: tagp[j, p, q, cls] = onehot[(2p+q)*128 + j, cls]
        tagp = np.ascontiguousarray(
            oh.reshape(npair, 2, 128, ncls).transpose(2, 0, 1, 3)
        ).astype(ml_dtypes.float8_e5m2)
        oha = np.ascontiguousarray(oh[:b].T)  # [ncls, b]
        in_maps.append({"x": xs, "tagp": tagp, "oha": oha,
                        "i128": i128, "i128b": i128b, "i8e5": i8e5})
    return in_maps


def kernel(embeddings, target):
    from concourse.bass_utils import run_bass_kernel_spmd

    nc = _get_nc()
    in_maps = make_in_maps(embeddings, target)
    res = run_bass_kernel_spmd(nc, in_maps, list(range(NCORES))).results
    loss = np.concatenate([np.asarray(res[c]["loss"]).ravel() for c in range(NCORES)])
    return np.float32(loss.sum())
